# revision 30
# baseline (speedup 1.0000x reference)
"""Bilateral filter (7x7, dilation 1) Trainium2 Bass kernel, v3.

Problem: input [2, 18, 1024, 1024] f32.
  filterable f = input[:, :8]; params p = input[:, 8:]
  logw(tap) = -sum_c p_c^2 (fn_c - f_c)^2 - p8^2 dx^2 - p9^2 dy^2
  out_c = sum_taps exp(logw) fn_c / sum_taps exp(logw),  c < 3

Sharding: 8 cores x (one batch quarter of 256 rows + 3 halo rows), as in
the baseline.  Out-of-image taps are killed by a 1e18 sentinel pad (the
quadratic form then underflows exp to +0).

Per-core layout (bf16):  macro = 64 output rows x full width.
  tiles put (channel-pair c in {0,1}, row r in 0..63) on partitions and
  (segment g in 0..3, x) on the free axis, so channel ci = 2g + c.
  Row shifts -> 7 DMA'd tile variants F[oy]; column shifts -> free-axis
  offsets (DVE needs in0/in1 offsets with equal parity, so the center
  operand comes from F[3] (odd offset 3) for odd j and from a separate
  even-phase tile Fce (offset 2) for even j).

Per tap: d = Fsh - Fc (DVE), d2 = Square(d) (ACT), q = (p^2)*d2 (DVE or
GPSIMD, flat), then PE matmuls with a -1 row-selector lhsT contract the
channels of q into PSUM (2 taps per bank, M=64 at partitions 0/64), plus
one matmul with lhsT = -(dx^2, dy^2) selector against the (p8^2, p9^2)
tile for the spatial term.  w = Exp(PSUM) on ACT.  acc_c and wsum are
accumulated over all taps by PE matmuls with a +1 selector against
WF_c = w * G_c (gathered neighbor tiles) and w itself.
"""

import sys

if "/opt/trn_rl_repo" not in sys.path:
    sys.path.insert(0, "/opt/trn_rl_repo")

import numpy as np
import ml_dtypes

import concourse.bass as bass
import concourse.mybir as mybir
from concourse.bacc import Bacc
from concourse.tile import TileContext

FP32 = mybir.dt.float32
BF16 = mybir.dt.bfloat16
FP8 = mybir.dt.float8e4
ALU = mybir.AluOpType
ACTF = mybir.ActivationFunctionType

B, C_ALL, H, W = 2, 18, 1024, 1024
CF, CO = 8, 3
KS, RAD = 7, 3
HC = 256                      # output rows per core
HIN = HC + 2 * RAD            # slab rows (262)
WP = W + 2 * RAD              # padded slab width (1030)
MR = 64                       # macro rows
NM = HC // MR                 # 4 macros
SEG = 4                       # channel-pair segments per tile
SW = W + 8                    # segment width in F tiles (1032, even)
SENT = 9.0
BF = ml_dtypes.bfloat16

_CACHED = {}

# tap pairing: 48 non-center taps in 24 pairs + (center, dead)
_TAPS = [(i, j) for i in range(KS) for j in range(KS) if not (i == RAD and j == RAD)]
PAIRS = [(_TAPS[2 * k], _TAPS[2 * k + 1]) for k in range(24)]
CENTER_PAIR_IDX = 24          # slot0 = center tap (w=1), slot1 = dead (w=0)

N_CONST = 19                  # SELN, SELP, 16 spatial matrices, SHIFTC


def _seg(ap, width=SW):
    return ap.rearrange("p (s x) -> p s x", s=SEG)


def build_consts8():
    c = np.zeros((1, 128, 64), dtype=np.float32)
    r = np.arange(64)
    for g in range(2):
        c[0, g * 64 + r, r] = -64.0
    return c.astype(ml_dtypes.float8_e4m3)


def build_consts():
    """[18, 128, 64] bf16: row-selector and spatial lhsT matrices."""
    c = np.zeros((N_CONST, 128, 64), dtype=np.float32)
    r = np.arange(64)
    for g in range(2):
        c[0, g * 64 + r, r] = -1.0          # SELN
        c[1, g * 64 + r, r] = 1.0           # SELP
    for a in range(4):                       # dx2 index
        for b in range(4):                   # dy2 index
            m = 2 + a * 4 + b
            c[m, 0 * 64 + r, r] = -float([0, 1, 4, 9][a])
            c[m, 1 * 64 + r, r] = -float([0, 1, 4, 9][b])
    c[18, 64 + r, r] = 1.0                   # SHIFTC: out[m] = in[m + 64]
    return c.astype(BF)


D2IDX = [3, 2, 1, 0, 1, 2, 3]


def build_nc():
    nc = Bacc()
    fil = nc.dram_tensor("fil", [CF, HIN, WP], BF16, kind="ExternalInput")
    prm = nc.dram_tensor("prm", [10, HC, W], BF16, kind="ExternalInput")
    cst = nc.dram_tensor("cst", [N_CONST, 128, 64], BF16, kind="ExternalInput")
    cst8 = nc.dram_tensor("cst8", [1, 128, 64], FP8, kind="ExternalInput")
    y = nc.dram_tensor("y", [CO, HC, W], FP32, kind="ExternalOutput")

    with TileContext(nc) as tc:
        with (
            tc.tile_pool(name="const", bufs=1) as kpool,
            tc.tile_pool(name="fpool", bufs=1) as fpool,
            tc.tile_pool(name="dpool", bufs=3) as dpool,
            tc.tile_pool(name="wpool", bufs=2) as wpool,
            tc.tile_pool(name="opool", bufs=2) as opool,
            tc.tile_pool(name="ppool", bufs=4, space="PSUM") as ppool,
        ):
            CST = kpool.tile([128, N_CONST * 64], BF16, name="CST")
            for m in range(N_CONST):
                nc.sync.dma_start(out=CST[:, m * 64:(m + 1) * 64], in_=cst[m])
            CST8 = kpool.tile([128, 64], FP8, name="CST8")
            nc.sync.dma_start(out=CST8[:], in_=cst8[0])
            SELN = CST8[:]
            SELP = CST[:, 64:128]
            SHIFTC = CST[:, 18 * 64:19 * 64]

            def SPAT(i, j):
                m = 2 + D2IDX[j] * 4 + D2IDX[i]
                return CST[:, m * 64:(m + 1) * 64]

            for mi in range(NM):
                _macro(nc, tc, fil, prm, y, kpool, fpool, dpool, wpool, opool,
                       ppool, mi, SELN, SELP, SPAT, SHIFTC)
    nc.compile()
    return nc


def _macro(nc, tc, fil, prm, y, kpool, fpool, dpool, wpool, opool, ppool, mi,
           SELN, SELP, SPAT, SHIFTC):
    r0 = mi * MR

    # ---- F tiles: 7 row-shift variants + even-phase center ----
    F = []
    for oy in range(KS):
        Ft = fpool.tile([128, SEG * SW], BF16, tag=f"F{oy}", name=f"F{oy}_{mi}")
        for g in range(SEG):
            for c in range(2):
                nc.sync.dma_start(
                    out=_seg(Ft[:])[c * 64:(c + 1) * 64, g, 0:WP],
                    in_=fil[2 * g + c, r0 + oy:r0 + oy + MR, 0:WP])
        F.append(Ft)
    Fce = fpool.tile([128, SEG * SW], BF16, tag="Fce", name=f"Fce_{mi}")
    for g in range(SEG):
        for c in range(2):
            nc.sync.dma_start(
                out=_seg(Fce[:])[c * 64:(c + 1) * 64, g, 0:WP - 1],
                in_=fil[2 * g + c, r0 + RAD:r0 + RAD + MR, 1:WP])
    # center views at offset 3 (odd phase, in F[3]) and offset 2 (even, Fce)
    Fc_o = _seg(F[RAD][:])[:, :, 3:3 + W]
    Fc_e = _seg(Fce[:])[:, :, 2:2 + W]

    # ---- R = p_c^2 (bf16), SXY = (p8^2, p9^2) ----
    Rw = fpool.tile([128, SEG * W], BF16, tag="R", name=f"R_{mi}")
    for g in range(SEG):
        for c in range(2):
            nc.sync.dma_start(
                out=Rw[c * 64:(c + 1) * 64, g * W:(g + 1) * W],
                in_=prm[2 * g + c, r0:r0 + MR, :])
    nc.vector.tensor_scalar_mul(Rw[:], Rw[:], 0.125)
    SXY = fpool.tile([128, W], BF16, tag="SXY", name=f"SXY_{mi}")
    for k in range(2):
        nc.sync.dma_start(out=SXY[k * 64:(k + 1) * 64, :],
                          in_=prm[8 + k, r0:r0 + MR, :])
    nc.vector.tensor_mul(SXY[:], SXY[:], SXY[:])

    # ---- PSUM accumulators: acc0,acc1 | acc2,wsum  (x2 N-halves) ----
    ACC = [ppool.tile([128, 512], FP32, tag="ACC", name=f"ACC{k}_{mi}")
           for k in range(4)]   # [A_h0, A_h1, B_h0, B_h1]

    nq = 0
    for pi, pair in enumerate(PAIRS + [((RAD, RAD), None)]):
        w_t = wpool.tile([128, W], BF16, tag="w", name=f"w_{mi}_{pi}", bufs=3)
        if pi == CENTER_PAIR_IDX:
            nc.gpsimd.memset(w_t[0:64, :], 1.0)
            nc.gpsimd.memset(w_t[64:128, :], 0.0)
            g_taps = (pair[0], pair[0])
        else:
            SP = [ppool.tile([128, 512], FP32, tag="SP", name=f"SP{h}_{mi}_{pi}",
                             bufs=4) for h in range(2)]
            d2s = []
            for sl, (i, j) in enumerate(pair):
                Fc = Fc_o if (j % 2 == 1) else Fc_e
                d = dpool.tile([128, SEG * W], BF16, tag="d",
                               name=f"d_{mi}_{pi}_{sl}", bufs=3)
                dv = d[:].rearrange("p (s x) -> p s x", s=SEG)
                nc.vector.tensor_sub(dv, _seg(F[i][:])[:, :, j:j + W], Fc)
                t8 = dpool.tile([128, SEG * W], FP8, tag="t8",
                                name=f"t8_{mi}_{pi}_{sl}", bufs=2)
                nc.vector.tensor_mul(t8[:], Rw[:], d[:])
                d2 = dpool.tile([128, SEG * W], FP8, tag="d2",
                                name=f"d2_{mi}_{pi}_{sl}", bufs=2)
                nc.scalar.activation(d2[:], t8[:], ACTF.Square)
                d2s.append(d2)
            for sl, (i, j) in enumerate(pair):
                d2 = d2s[sl]
                nq += 1
                pos = slice(sl * 64, sl * 64 + 64)
                for h in range(2):
                    for g in range(SEG):
                        nc.tensor.matmul(
                            SP[h][pos, :], SELN,
                            d2[:, g * W + h * 512:g * W + h * 512 + 512],
                            start=(g == 0), stop=False, skip_group_check=True)
                    nc.tensor.matmul(
                        SP[h][pos, :], SPAT(i, j),
                        SXY[:, h * 512:h * 512 + 512],
                        start=False, stop=True, skip_group_check=True)
            for h in range(2):
                nc.scalar.activation(w_t[:, h * 512:h * 512 + 512], SP[h][:],
                                     ACTF.Exp)
            g_taps = pair

        # ---- gather tiles + weighted accumulation ----
        first = pi == 0
        last = pi == CENTER_PAIR_IDX
        G3 = wpool.tile([128, CO * W], BF16, tag="G3", name=f"G3_{mi}_{pi}",
                        bufs=3)
        for sl in range(2):
            i, j = g_taps[sl]
            for c in range(CO):
                nc.sync.dma_start(
                    out=G3[sl * 64:sl * 64 + 64, c * W:(c + 1) * W],
                    in_=fil[c, r0 + i:r0 + i + MR, j:j + W])
        WFs = []
        for c in range(CO):
            WF = wpool.tile([128, W], BF16, tag=f"WF{c}",
                            name=f"WF{c}_{mi}_{pi}", bufs=3)
            nc.vector.tensor_mul(WF[:], w_t[:], G3[:, c * W:(c + 1) * W])
            WFs.append(WF)
        for h in range(2):
            hs = slice(h * 512, h * 512 + 512)
            nc.tensor.matmul(ACC[h][0:64, :], SELP, WFs[0][:, hs],
                             start=first, stop=last, skip_group_check=True)
            nc.tensor.matmul(ACC[h][64:128, :], SELP, WFs[1][:, hs],
                             start=first, stop=last, skip_group_check=True)
            nc.tensor.matmul(ACC[2 + h][0:64, :], SELP, WFs[2][:, hs],
                             start=first, stop=last, skip_group_check=True)
            nc.tensor.matmul(ACC[2 + h][64:128, :], SELP, w_t[:, hs],
                             start=first, stop=last, skip_group_check=True)

    # ---- out = acc / wsum ----
    # wsum lives at partitions 64:128 of ACC[2+h]; DMA-replicate it to both
    # partition halves so every elementwise op sees matching base partitions.
    for h in range(2):
        hs = slice(h * 512, h * 512 + 512)
        AS = opool.tile([128, 512], BF16, tag="as", name=f"as_{mi}_{h}")
        nc.scalar.copy(AS[:], ACC[2 + h][:])
        TMP = ppool.tile([128, 512], FP32, tag="SP", name=f"wtmp_{mi}_{h}",
                         bufs=4)
        nc.tensor.matmul(TMP[0:64, :], SHIFTC, AS[:], start=True, stop=True,
                         skip_group_check=True)
        RW = opool.tile([128, 512], FP32, tag="rw", name=f"rw_{mi}_{h}")
        nc.vector.reciprocal(RW[0:64, :], TMP[0:64, :])
        nc.vector.reciprocal(RW[64:128, :], ACC[2 + h][64:128, :])
        o = opool.tile([128, 512], FP32, tag="o", name=f"o_{mi}_{h}")
        o2 = opool.tile([128, 512], FP32, tag="o2", name=f"o2_{mi}_{h}")
        nc.vector.tensor_mul(o[0:64, :], ACC[h][0:64, :], RW[0:64, :])
        nc.vector.tensor_mul(o[64:128, :], ACC[h][64:128, :], RW[64:128, :])
        nc.vector.tensor_mul(o2[0:64, :], ACC[2 + h][0:64, :], RW[0:64, :])
        nc.sync.dma_start(out=y[0, r0:r0 + MR, hs], in_=o[0:64, :])
        nc.sync.dma_start(out=y[1, r0:r0 + MR, hs], in_=o[64:128, :])
        nc.sync.dma_start(out=y[2, r0:r0 + MR, hs], in_=o2[0:64, :])


def shard_inputs(input):
    """input [2,18,1024,1024] f32 -> per-core {fil, prm, cst} bf16."""
    input = np.asarray(input, dtype=np.float32)
    cst = build_consts()
    cst8v = build_consts8()
    fil_all = input[:, :CF]
    prm_all = input[:, CF:]
    in_maps = []
    for core in range(8):
        b, q = divmod(core, 4)
        r0 = q * HC
        slab = np.full((CF, HIN, WP), SENT, dtype=np.float32)
        s_lo = max(r0 - RAD, 0)
        s_hi = min(r0 + HC + RAD, H)
        slab[:, s_lo - (r0 - RAD):s_hi - (r0 - RAD), RAD:RAD + W] = \
            fil_all[b, :, s_lo:s_hi, :]
        prm = np.abs(prm_all[b, :, r0:r0 + HC, :])
        in_maps.append({
            "fil": np.ascontiguousarray(slab.astype(BF)),
            "prm": np.ascontiguousarray(prm.astype(BF)),
            "cst": cst,
            "cst8": cst8v,
        })
    return in_maps


def assemble(results):
    out = np.empty((B, CO, H, W), dtype=np.float32)
    for core in range(8):
        b, q = divmod(core, 4)
        out[b, :, q * HC:(q + 1) * HC, :] = results[core]["y"]
    return out


def kernel(input):
    from concourse.bass_utils import run_bass_kernel_spmd

    if "nc" not in _CACHED:
        _CACHED["nc"] = build_nc()
    in_maps = shard_inputs(input)
    res = run_bass_kernel_spmd(_CACHED["nc"], in_maps, list(range(8)))
    return assemble(res.results)


# revision 31
# speedup vs baseline: 1.2111x; 1.2111x over previous
"""Bilateral filter (7x7, dilation 1) Trainium2 Bass kernel, v3.

Problem: input [2, 18, 1024, 1024] f32.
  filterable f = input[:, :8]; params p = input[:, 8:]
  logw(tap) = -sum_c p_c^2 (fn_c - f_c)^2 - p8^2 dx^2 - p9^2 dy^2
  out_c = sum_taps exp(logw) fn_c / sum_taps exp(logw),  c < 3

Sharding: 8 cores x (one batch quarter of 256 rows + 3 halo rows), as in
the baseline.  Out-of-image taps are killed by a 1e18 sentinel pad (the
quadratic form then underflows exp to +0).

Per-core layout (bf16):  macro = 64 output rows x full width.
  tiles put (channel-pair c in {0,1}, row r in 0..63) on partitions and
  (segment g in 0..3, x) on the free axis, so channel ci = 2g + c.
  Row shifts -> 7 DMA'd tile variants F[oy]; column shifts -> free-axis
  offsets (DVE needs in0/in1 offsets with equal parity, so the center
  operand comes from F[3] (odd offset 3) for odd j and from a separate
  even-phase tile Fce (offset 2) for even j).

Per tap: d = Fsh - Fc (DVE), d2 = Square(d) (ACT), q = (p^2)*d2 (DVE or
GPSIMD, flat), then PE matmuls with a -1 row-selector lhsT contract the
channels of q into PSUM (2 taps per bank, M=64 at partitions 0/64), plus
one matmul with lhsT = -(dx^2, dy^2) selector against the (p8^2, p9^2)
tile for the spatial term.  w = Exp(PSUM) on ACT.  acc_c and wsum are
accumulated over all taps by PE matmuls with a +1 selector against
WF_c = w * G_c (gathered neighbor tiles) and w itself.
"""

import sys

if "/opt/trn_rl_repo" not in sys.path:
    sys.path.insert(0, "/opt/trn_rl_repo")

import numpy as np
import ml_dtypes

import concourse.bass as bass
import concourse.mybir as mybir
from concourse.bacc import Bacc
from concourse.tile import TileContext

FP32 = mybir.dt.float32
BF16 = mybir.dt.bfloat16
ALU = mybir.AluOpType
ACTF = mybir.ActivationFunctionType

B, C_ALL, H, W = 2, 18, 1024, 1024
CF, CO = 8, 3
KS, RAD = 7, 3
HC = 256                      # output rows per core
HIN = HC + 2 * RAD            # slab rows (262)
WP = W + 2 * RAD              # padded slab width (1030)
MR = 64                       # macro rows
NM = HC // MR                 # 4 macros
SEG = 4                       # channel-pair segments per tile
SW = W + 8                    # segment width in F tiles (1032, even)
SENT = 1.0e18
BF = ml_dtypes.bfloat16

_CACHED = {}

# tap pairing: 48 non-center taps in 24 pairs + (center, dead)
_TAPS = [(i, j) for i in range(KS) for j in range(KS) if not (i == RAD and j == RAD)]
PAIRS = [(_TAPS[2 * k], _TAPS[2 * k + 1]) for k in range(24)]
CENTER_PAIR_IDX = 24          # slot0 = center tap (w=1), slot1 = dead (w=0)

N_CONST = 19                  # SELN, SELP, 16 spatial matrices, SHIFTC


def _seg(ap, width=SW):
    return ap.rearrange("p (s x) -> p s x", s=SEG)


def build_consts():
    """[18, 128, 64] bf16: row-selector and spatial lhsT matrices."""
    c = np.zeros((N_CONST, 128, 64), dtype=np.float32)
    r = np.arange(64)
    for g in range(2):
        c[0, g * 64 + r, r] = -1.0          # SELN
        c[1, g * 64 + r, r] = 1.0           # SELP
    for a in range(4):                       # dx2 index
        for b in range(4):                   # dy2 index
            m = 2 + a * 4 + b
            c[m, 0 * 64 + r, r] = -float([0, 1, 4, 9][a])
            c[m, 1 * 64 + r, r] = -float([0, 1, 4, 9][b])
    c[18, 64 + r, r] = 1.0                   # SHIFTC: out[m] = in[m + 64]
    return c.astype(BF)


D2IDX = [3, 2, 1, 0, 1, 2, 3]


def build_nc():
    nc = Bacc()
    fil = nc.dram_tensor("fil", [CF, HIN, WP], BF16, kind="ExternalInput")
    prm = nc.dram_tensor("prm", [10, HC, W], BF16, kind="ExternalInput")
    cst = nc.dram_tensor("cst", [N_CONST, 128, 64], BF16, kind="ExternalInput")
    y = nc.dram_tensor("y", [CO, HC, W], FP32, kind="ExternalOutput")

    with TileContext(nc) as tc:
        with (
            tc.tile_pool(name="const", bufs=1) as kpool,
            tc.tile_pool(name="fpool", bufs=1) as fpool,
            tc.tile_pool(name="dpool", bufs=3) as dpool,
            tc.tile_pool(name="wpool", bufs=2) as wpool,
            tc.tile_pool(name="opool", bufs=2) as opool,
            tc.tile_pool(name="ppool", bufs=4, space="PSUM") as ppool,
        ):
            CST = kpool.tile([128, N_CONST * 64], BF16, name="CST")
            for m in range(N_CONST):
                nc.sync.dma_start(out=CST[:, m * 64:(m + 1) * 64], in_=cst[m])
            SELN = CST[:, 0:64]
            SELP = CST[:, 64:128]
            SHIFTC = CST[:, 18 * 64:19 * 64]

            def SPAT(i, j):
                m = 2 + D2IDX[j] * 4 + D2IDX[i]
                return CST[:, m * 64:(m + 1) * 64]

            for mi in range(NM):
                _macro(nc, tc, fil, prm, y, kpool, fpool, dpool, wpool, opool,
                       ppool, mi, SELN, SELP, SPAT, SHIFTC)
    nc.compile()
    return nc


def _macro(nc, tc, fil, prm, y, kpool, fpool, dpool, wpool, opool, ppool, mi,
           SELN, SELP, SPAT, SHIFTC):
    r0 = mi * MR

    # ---- F tiles: 7 row-shift variants + even-phase center ----
    F = []
    for oy in range(KS):
        Ft = fpool.tile([128, SEG * SW], BF16, tag=f"F{oy}", name=f"F{oy}_{mi}")
        for g in range(SEG):
            for c in range(2):
                nc.sync.dma_start(
                    out=_seg(Ft[:])[c * 64:(c + 1) * 64, g, 0:WP],
                    in_=fil[2 * g + c, r0 + oy:r0 + oy + MR, 0:WP])
        F.append(Ft)
    Fce = fpool.tile([128, SEG * SW], BF16, tag="Fce", name=f"Fce_{mi}")
    for g in range(SEG):
        for c in range(2):
            nc.sync.dma_start(
                out=_seg(Fce[:])[c * 64:(c + 1) * 64, g, 0:WP - 1],
                in_=fil[2 * g + c, r0 + RAD:r0 + RAD + MR, 1:WP])
    # center views at offset 3 (odd phase, in F[3]) and offset 2 (even, Fce)
    Fc_o = _seg(F[RAD][:])[:, :, 3:3 + W]
    Fc_e = _seg(Fce[:])[:, :, 2:2 + W]

    # ---- R = p_c^2 (bf16), SXY = (p8^2, p9^2) ----
    Rw = fpool.tile([128, SEG * W], BF16, tag="R", name=f"R_{mi}")
    for g in range(SEG):
        for c in range(2):
            nc.sync.dma_start(
                out=Rw[c * 64:(c + 1) * 64, g * W:(g + 1) * W],
                in_=prm[2 * g + c, r0:r0 + MR, :])
    nc.vector.tensor_mul(Rw[:], Rw[:], Rw[:])
    SXY = fpool.tile([128, W], BF16, tag="SXY", name=f"SXY_{mi}")
    for k in range(2):
        nc.sync.dma_start(out=SXY[k * 64:(k + 1) * 64, :],
                          in_=prm[8 + k, r0:r0 + MR, :])
    nc.vector.tensor_mul(SXY[:], SXY[:], SXY[:])

    # ---- PSUM accumulators: acc0,acc1 | acc2,wsum  (x2 N-halves) ----
    ACC = [ppool.tile([128, 512], FP32, tag="ACC", name=f"ACC{k}_{mi}")
           for k in range(4)]   # [A_h0, A_h1, B_h0, B_h1]

    nq = 0
    for pi, pair in enumerate(PAIRS + [((RAD, RAD), None)]):
        w_t = wpool.tile([128, W], BF16, tag="w", name=f"w_{mi}_{pi}", bufs=3)
        if pi == CENTER_PAIR_IDX:
            nc.gpsimd.memset(w_t[0:64, :], 1.0)
            nc.gpsimd.memset(w_t[64:128, :], 0.0)
            g_taps = (pair[0], pair[0])
        else:
            SP = [ppool.tile([128, 512], FP32, tag="SP", name=f"SP{h}_{mi}_{pi}",
                             bufs=4) for h in range(2)]
            ds, d2s = [], []
            for sl, (i, j) in enumerate(pair):
                Fc = Fc_o if (j % 2 == 1) else Fc_e
                d = dpool.tile([128, SEG * W], BF16, tag="d",
                               name=f"d_{mi}_{pi}_{sl}", bufs=3)
                dv = d[:].rearrange("p (s x) -> p s x", s=SEG)
                nc.vector.tensor_sub(dv, _seg(F[i][:])[:, :, j:j + W], Fc)
                ds.append(d)
            for sl in range(2):
                d2 = dpool.tile([128, SEG * W], BF16, tag="d2",
                                name=f"d2_{mi}_{pi}_{sl}", bufs=2)
                nc.scalar.activation(d2[:], ds[sl][:], ACTF.Square)
                d2s.append(d2)
            for sl, (i, j) in enumerate(pair):
                d2 = d2s[sl]
                nc.vector.tensor_mul(d2[:], Rw[:], d2[:])
                nq += 1
                pos = slice(sl * 64, sl * 64 + 64)
                for h in range(2):
                    for g in range(SEG):
                        nc.tensor.matmul(
                            SP[h][pos, :], SELN,
                            d2[:, g * W + h * 512:g * W + h * 512 + 512],
                            start=(g == 0), stop=False, skip_group_check=True)
                    nc.tensor.matmul(
                        SP[h][pos, :], SPAT(i, j),
                        SXY[:, h * 512:h * 512 + 512],
                        start=False, stop=True, skip_group_check=True)
            for h in range(2):
                nc.scalar.activation(w_t[:, h * 512:h * 512 + 512], SP[h][:],
                                     ACTF.Exp)
            g_taps = pair

        # ---- gather tiles + weighted accumulation ----
        first = pi == 0
        last = pi == CENTER_PAIR_IDX
        G3 = wpool.tile([128, CO * W], BF16, tag="G3", name=f"G3_{mi}_{pi}",
                        bufs=3)
        for sl in range(2):
            i, j = g_taps[sl]
            for c in range(CO):
                nc.sync.dma_start(
                    out=G3[sl * 64:sl * 64 + 64, c * W:(c + 1) * W],
                    in_=fil[c, r0 + i:r0 + i + MR, j:j + W])
        WFs = []
        for c in range(CO):
            WF = wpool.tile([128, W], BF16, tag=f"WF{c}",
                            name=f"WF{c}_{mi}_{pi}", bufs=3)
            nc.vector.tensor_mul(WF[:], w_t[:], G3[:, c * W:(c + 1) * W])
            WFs.append(WF)
        for h in range(2):
            hs = slice(h * 512, h * 512 + 512)
            nc.tensor.matmul(ACC[h][0:64, :], SELP, WFs[0][:, hs],
                             start=first, stop=last, skip_group_check=True)
            nc.tensor.matmul(ACC[h][64:128, :], SELP, WFs[1][:, hs],
                             start=first, stop=last, skip_group_check=True)
            nc.tensor.matmul(ACC[2 + h][0:64, :], SELP, WFs[2][:, hs],
                             start=first, stop=last, skip_group_check=True)
            nc.tensor.matmul(ACC[2 + h][64:128, :], SELP, w_t[:, hs],
                             start=first, stop=last, skip_group_check=True)

    # ---- out = acc / wsum ----
    # wsum lives at partitions 64:128 of ACC[2+h]; DMA-replicate it to both
    # partition halves so every elementwise op sees matching base partitions.
    for h in range(2):
        hs = slice(h * 512, h * 512 + 512)
        AS = opool.tile([128, 512], BF16, tag="as", name=f"as_{mi}_{h}")
        nc.scalar.copy(AS[:], ACC[2 + h][:])
        TMP = ppool.tile([128, 512], FP32, tag="SP", name=f"wtmp_{mi}_{h}",
                         bufs=4)
        nc.tensor.matmul(TMP[0:64, :], SHIFTC, AS[:], start=True, stop=True,
                         skip_group_check=True)
        RW = opool.tile([128, 512], FP32, tag="rw", name=f"rw_{mi}_{h}")
        nc.vector.reciprocal(RW[0:64, :], TMP[0:64, :])
        nc.vector.reciprocal(RW[64:128, :], ACC[2 + h][64:128, :])
        o = opool.tile([128, 512], FP32, tag="o", name=f"o_{mi}_{h}")
        o2 = opool.tile([128, 512], FP32, tag="o2", name=f"o2_{mi}_{h}")
        nc.vector.tensor_mul(o[0:64, :], ACC[h][0:64, :], RW[0:64, :])
        nc.vector.tensor_mul(o[64:128, :], ACC[h][64:128, :], RW[64:128, :])
        nc.vector.tensor_mul(o2[0:64, :], ACC[2 + h][0:64, :], RW[0:64, :])
        nc.sync.dma_start(out=y[0, r0:r0 + MR, hs], in_=o[0:64, :])
        nc.sync.dma_start(out=y[1, r0:r0 + MR, hs], in_=o[64:128, :])
        nc.sync.dma_start(out=y[2, r0:r0 + MR, hs], in_=o2[0:64, :])


def shard_inputs(input):
    """input [2,18,1024,1024] f32 -> per-core {fil, prm, cst} bf16."""
    input = np.asarray(input, dtype=np.float32)
    cst = build_consts()
    fil_all = input[:, :CF]
    prm_all = input[:, CF:]
    in_maps = []
    for core in range(8):
        b, q = divmod(core, 4)
        r0 = q * HC
        slab = np.full((CF, HIN, WP), SENT, dtype=np.float32)
        s_lo = max(r0 - RAD, 0)
        s_hi = min(r0 + HC + RAD, H)
        slab[:, s_lo - (r0 - RAD):s_hi - (r0 - RAD), RAD:RAD + W] = \
            fil_all[b, :, s_lo:s_hi, :]
        prm = prm_all[b, :, r0:r0 + HC, :]
        in_maps.append({
            "fil": np.ascontiguousarray(slab.astype(BF)),
            "prm": np.ascontiguousarray(prm.astype(BF)),
            "cst": cst,
        })
    return in_maps


def assemble(results):
    out = np.empty((B, CO, H, W), dtype=np.float32)
    for core in range(8):
        b, q = divmod(core, 4)
        out[b, :, q * HC:(q + 1) * HC, :] = results[core]["y"]
    return out


def kernel(input):
    from concourse.bass_utils import run_bass_kernel_spmd

    if "nc" not in _CACHED:
        _CACHED["nc"] = build_nc()
    in_maps = shard_inputs(input)
    res = run_bass_kernel_spmd(_CACHED["nc"], in_maps, list(range(8)))
    return assemble(res.results)


# revision 32
# speedup vs baseline: 1.3403x; 1.1066x over previous
"""Bilateral filter (7x7, dilation 1) Trainium2 Bass kernel, v3.

Problem: input [2, 18, 1024, 1024] f32.
  filterable f = input[:, :8]; params p = input[:, 8:]
  logw(tap) = -sum_c p_c^2 (fn_c - f_c)^2 - p8^2 dx^2 - p9^2 dy^2
  out_c = sum_taps exp(logw) fn_c / sum_taps exp(logw),  c < 3

Sharding: 8 cores x (one batch quarter of 256 rows + 3 halo rows), as in
the baseline.  Out-of-image taps are killed by a 1e18 sentinel pad (the
quadratic form then underflows exp to +0).

Per-core layout (bf16):  macro = 64 output rows x full width.
  tiles put (channel-pair c in {0,1}, row r in 0..63) on partitions and
  (segment g in 0..3, x) on the free axis, so channel ci = 2g + c.
  Row shifts -> 7 DMA'd tile variants F[oy]; column shifts -> free-axis
  offsets (DVE needs in0/in1 offsets with equal parity, so the center
  operand comes from F[3] (odd offset 3) for odd j and from a separate
  even-phase tile Fce (offset 2) for even j).

Per tap: d = Fsh - Fc (DVE), d2 = Square(d) (ACT), q = (p^2)*d2 (DVE or
GPSIMD, flat), then PE matmuls with a -1 row-selector lhsT contract the
channels of q into PSUM (2 taps per bank, M=64 at partitions 0/64), plus
one matmul with lhsT = -(dx^2, dy^2) selector against the (p8^2, p9^2)
tile for the spatial term.  w = Exp(PSUM) on ACT.  acc_c and wsum are
accumulated over all taps by PE matmuls with a +1 selector against
WF_c = w * G_c (gathered neighbor tiles) and w itself.
"""

import sys

if "/opt/trn_rl_repo" not in sys.path:
    sys.path.insert(0, "/opt/trn_rl_repo")

import numpy as np
import ml_dtypes

import concourse.bass as bass
import concourse.mybir as mybir
from concourse.bacc import Bacc
from concourse.tile import TileContext

FP32 = mybir.dt.float32
BF16 = mybir.dt.bfloat16
FP8 = mybir.dt.float8e4
ALU = mybir.AluOpType
ACTF = mybir.ActivationFunctionType

B, C_ALL, H, W = 2, 18, 1024, 1024
CF, CO = 8, 3
KS, RAD = 7, 3
HC = 256                      # output rows per core
HIN = HC + 2 * RAD            # slab rows (262)
WP = W + 2 * RAD              # padded slab width (1030)
MR = 64                       # macro rows
NM = HC // MR                 # 4 macros
SEG = 4                       # channel-pair segments per tile
SW = W + 8                    # segment width in F tiles (1032, even)
SENT = 9.0
BF = ml_dtypes.bfloat16

_CACHED = {}

# tap pairing: 48 non-center taps in 24 pairs + (center, dead)
_TAPS = [(i, j) for i in range(KS) for j in range(KS) if not (i == RAD and j == RAD)]
PAIRS = [(_TAPS[2 * k], _TAPS[2 * k + 1]) for k in range(24)]
CENTER_PAIR_IDX = 24          # slot0 = center tap (w=1), slot1 = dead (w=0)

N_CONST = 19                  # SELN, SELP, 16 spatial matrices, SHIFTC


def _seg(ap, width=SW):
    return ap.rearrange("p (s x) -> p s x", s=SEG)


def build_consts8():
    c = np.zeros((1, 128, 64), dtype=np.float32)
    r = np.arange(64)
    for g in range(2):
        c[0, g * 64 + r, r] = -64.0
    return c.astype(ml_dtypes.float8_e4m3)


def build_consts():
    """[18, 128, 64] bf16: row-selector and spatial lhsT matrices."""
    c = np.zeros((N_CONST, 128, 64), dtype=np.float32)
    r = np.arange(64)
    for g in range(2):
        c[0, g * 64 + r, r] = -1.0          # SELN
        c[1, g * 64 + r, r] = 1.0           # SELP
    for a in range(4):                       # dx2 index
        for b in range(4):                   # dy2 index
            m = 2 + a * 4 + b
            c[m, 0 * 64 + r, r] = -float([0, 1, 4, 9][a])
            c[m, 1 * 64 + r, r] = -float([0, 1, 4, 9][b])
    c[18, 64 + r, r] = 1.0                   # SHIFTC: out[m] = in[m + 64]
    return c.astype(BF)


D2IDX = [3, 2, 1, 0, 1, 2, 3]


def build_nc():
    nc = Bacc()
    fil = nc.dram_tensor("fil", [CF, HIN, WP], BF16, kind="ExternalInput")
    prm = nc.dram_tensor("prm", [10, HC, W], BF16, kind="ExternalInput")
    cst = nc.dram_tensor("cst", [N_CONST, 128, 64], BF16, kind="ExternalInput")
    cst8 = nc.dram_tensor("cst8", [1, 128, 64], FP8, kind="ExternalInput")
    y = nc.dram_tensor("y", [CO, HC, W], FP32, kind="ExternalOutput")

    with TileContext(nc) as tc:
        with (
            tc.tile_pool(name="const", bufs=1) as kpool,
            tc.tile_pool(name="fpool", bufs=1) as fpool,
            tc.tile_pool(name="dpool", bufs=3) as dpool,
            tc.tile_pool(name="wpool", bufs=2) as wpool,
            tc.tile_pool(name="opool", bufs=2) as opool,
            tc.tile_pool(name="ppool", bufs=4, space="PSUM") as ppool,
        ):
            CST = kpool.tile([128, N_CONST * 64], BF16, name="CST")
            for m in range(N_CONST):
                nc.sync.dma_start(out=CST[:, m * 64:(m + 1) * 64], in_=cst[m])
            CST8 = kpool.tile([128, 64], FP8, name="CST8")
            nc.sync.dma_start(out=CST8[:], in_=cst8[0])
            SELN = CST8[:]
            SELP = CST[:, 64:128]
            SHIFTC = CST[:, 18 * 64:19 * 64]

            def SPAT(i, j):
                m = 2 + D2IDX[j] * 4 + D2IDX[i]
                return CST[:, m * 64:(m + 1) * 64]

            for mi in range(NM):
                _macro(nc, tc, fil, prm, y, kpool, fpool, dpool, wpool, opool,
                       ppool, mi, SELN, SELP, SPAT, SHIFTC)
    nc.compile()
    return nc


def _macro(nc, tc, fil, prm, y, kpool, fpool, dpool, wpool, opool, ppool, mi,
           SELN, SELP, SPAT, SHIFTC):
    r0 = mi * MR

    # ---- F tiles: 7 row-shift variants + even-phase center ----
    F = []
    for oy in range(KS):
        Ft = fpool.tile([128, SEG * SW], BF16, tag=f"F{oy}", name=f"F{oy}_{mi}")
        for g in range(SEG):
            for c in range(2):
                nc.sync.dma_start(
                    out=_seg(Ft[:])[c * 64:(c + 1) * 64, g, 0:WP],
                    in_=fil[2 * g + c, r0 + oy:r0 + oy + MR, 0:WP])
        F.append(Ft)
    Fce = fpool.tile([128, SEG * SW], BF16, tag="Fce", name=f"Fce_{mi}")
    for g in range(SEG):
        for c in range(2):
            nc.sync.dma_start(
                out=_seg(Fce[:])[c * 64:(c + 1) * 64, g, 0:WP - 1],
                in_=fil[2 * g + c, r0 + RAD:r0 + RAD + MR, 1:WP])
    # center views at offset 3 (odd phase, in F[3]) and offset 2 (even, Fce)
    Fc_o = _seg(F[RAD][:])[:, :, 3:3 + W]
    Fc_e = _seg(Fce[:])[:, :, 2:2 + W]

    # ---- R = p_c^2 (bf16), SXY = (p8^2, p9^2) ----
    Rw = fpool.tile([128, SEG * W], BF16, tag="R", name=f"R_{mi}")
    for g in range(SEG):
        for c in range(2):
            nc.sync.dma_start(
                out=Rw[c * 64:(c + 1) * 64, g * W:(g + 1) * W],
                in_=prm[2 * g + c, r0:r0 + MR, :])
    SXY = fpool.tile([128, W], BF16, tag="SXY", name=f"SXY_{mi}")
    for k in range(2):
        nc.sync.dma_start(out=SXY[k * 64:(k + 1) * 64, :],
                          in_=prm[8 + k, r0:r0 + MR, :])

    # ---- PSUM accumulators: acc0,acc1 | acc2,wsum  (x2 N-halves) ----
    ACC = [ppool.tile([128, 512], FP32, tag="ACC", name=f"ACC{k}_{mi}")
           for k in range(4)]   # [A_h0, A_h1, B_h0, B_h1]

    nq = 0
    for pi, pair in enumerate(PAIRS + [((RAD, RAD), None)]):
        w_t = wpool.tile([128, W], BF16, tag="w", name=f"w_{mi}_{pi}", bufs=3)
        if pi == CENTER_PAIR_IDX:
            nc.gpsimd.memset(w_t[0:64, :], 1.0)
            nc.gpsimd.memset(w_t[64:128, :], 0.0)
            g_taps = (pair[0], pair[0])
        else:
            SP = [ppool.tile([128, 512], FP32, tag="SP", name=f"SP{h}_{mi}_{pi}",
                             bufs=4) for h in range(2)]
            d2s = []
            for sl, (i, j) in enumerate(pair):
                Fc = Fc_o if (j % 2 == 1) else Fc_e
                d = dpool.tile([128, SEG * W], BF16, tag="d",
                               name=f"d_{mi}_{pi}_{sl}", bufs=2)
                dv = d[:].rearrange("p (s x) -> p s x", s=SEG)
                nc.vector.tensor_sub(dv, _seg(F[i][:])[:, :, j:j + W], Fc)
                t8 = dpool.tile([128, SEG * W], BF16, tag="t8",
                                name=f"t8_{mi}_{pi}_{sl}", bufs=2)
                nc.vector.tensor_mul(t8[:], Rw[:], d[:])
                d2 = dpool.tile([128, SEG * W], FP8, tag="d2",
                                name=f"d2_{mi}_{pi}_{sl}", bufs=2)
                nc.scalar.activation(d2[:], t8[:], ACTF.Square)
                d2s.append(d2)
            for sl, (i, j) in enumerate(pair):
                d2 = d2s[sl]
                nq += 1
                pos = slice(sl * 64, sl * 64 + 64)
                for h in range(2):
                    for g in range(SEG):
                        nc.tensor.matmul(
                            SP[h][pos, :], SELN,
                            d2[:, g * W + h * 512:g * W + h * 512 + 512],
                            start=(g == 0), stop=False, skip_group_check=True)
                    nc.tensor.matmul(
                        SP[h][pos, :], SPAT(i, j),
                        SXY[:, h * 512:h * 512 + 512],
                        start=False, stop=True, skip_group_check=True)
            for h in range(2):
                nc.scalar.activation(w_t[:, h * 512:h * 512 + 512], SP[h][:],
                                     ACTF.Exp)
            g_taps = pair

        # ---- gather tiles + weighted accumulation ----
        first = pi == 0
        last = pi == CENTER_PAIR_IDX
        G3 = wpool.tile([128, CO * W], BF16, tag="G3", name=f"G3_{mi}_{pi}",
                        bufs=3)
        for sl in range(2):
            i, j = g_taps[sl]
            for c in range(CO):
                nc.sync.dma_start(
                    out=G3[sl * 64:sl * 64 + 64, c * W:(c + 1) * W],
                    in_=fil[c, r0 + i:r0 + i + MR, j:j + W])
        WFs = []
        for c in range(CO):
            WF = wpool.tile([128, W], BF16, tag=f"WF{c}",
                            name=f"WF{c}_{mi}_{pi}", bufs=3)
            nc.vector.tensor_mul(WF[:], w_t[:], G3[:, c * W:(c + 1) * W])
            WFs.append(WF)
        for h in range(2):
            hs = slice(h * 512, h * 512 + 512)
            nc.tensor.matmul(ACC[h][0:64, :], SELP, WFs[0][:, hs],
                             start=first, stop=last, skip_group_check=True)
            nc.tensor.matmul(ACC[h][64:128, :], SELP, WFs[1][:, hs],
                             start=first, stop=last, skip_group_check=True)
            nc.tensor.matmul(ACC[2 + h][0:64, :], SELP, WFs[2][:, hs],
                             start=first, stop=last, skip_group_check=True)
            nc.tensor.matmul(ACC[2 + h][64:128, :], SELP, w_t[:, hs],
                             start=first, stop=last, skip_group_check=True)

    # ---- out = acc / wsum ----
    # wsum lives at partitions 64:128 of ACC[2+h]; DMA-replicate it to both
    # partition halves so every elementwise op sees matching base partitions.
    for h in range(2):
        hs = slice(h * 512, h * 512 + 512)
        AS = opool.tile([128, 512], BF16, tag="as", name=f"as_{mi}_{h}")
        nc.scalar.copy(AS[:], ACC[2 + h][:])
        TMP = ppool.tile([128, 512], FP32, tag="SP", name=f"wtmp_{mi}_{h}",
                         bufs=4)
        nc.tensor.matmul(TMP[0:64, :], SHIFTC, AS[:], start=True, stop=True,
                         skip_group_check=True)
        RW = opool.tile([128, 512], FP32, tag="rw", name=f"rw_{mi}_{h}")
        nc.vector.reciprocal(RW[0:64, :], TMP[0:64, :])
        nc.vector.reciprocal(RW[64:128, :], ACC[2 + h][64:128, :])
        o = opool.tile([128, 512], FP32, tag="o", name=f"o_{mi}_{h}")
        o2 = opool.tile([128, 512], FP32, tag="o2", name=f"o2_{mi}_{h}")
        nc.vector.tensor_mul(o[0:64, :], ACC[h][0:64, :], RW[0:64, :])
        nc.vector.tensor_mul(o[64:128, :], ACC[h][64:128, :], RW[64:128, :])
        nc.vector.tensor_mul(o2[0:64, :], ACC[2 + h][0:64, :], RW[0:64, :])
        nc.sync.dma_start(out=y[0, r0:r0 + MR, hs], in_=o[0:64, :])
        nc.sync.dma_start(out=y[1, r0:r0 + MR, hs], in_=o[64:128, :])
        nc.sync.dma_start(out=y[2, r0:r0 + MR, hs], in_=o2[0:64, :])


def shard_inputs(input):
    """input [2,18,1024,1024] f32 -> per-core {fil, prm, cst} bf16."""
    input = np.asarray(input, dtype=np.float32)
    cst = build_consts()
    cst8v = build_consts8()
    fil_all = input[:, :CF]
    prm_all = input[:, CF:]
    in_maps = []
    for core in range(8):
        b, q = divmod(core, 4)
        r0 = q * HC
        slab = np.full((CF, HIN, WP), SENT, dtype=np.float32)
        s_lo = max(r0 - RAD, 0)
        s_hi = min(r0 + HC + RAD, H)
        slab[:, s_lo - (r0 - RAD):s_hi - (r0 - RAD), RAD:RAD + W] = \
            fil_all[b, :, s_lo:s_hi, :]
        prm = prm_all[b, :, r0:r0 + HC, :].astype(np.float32).copy()
        prm[0:8] = np.abs(prm[0:8]) * 0.125
        prm[8:10] = prm[8:10] * prm[8:10]
        in_maps.append({
            "fil": np.ascontiguousarray(slab.astype(BF)),
            "prm": np.ascontiguousarray(prm.astype(BF)),
            "cst": cst,
            "cst8": cst8v,
        })
    return in_maps


def assemble(results):
    out = np.empty((B, CO, H, W), dtype=np.float32)
    for core in range(8):
        b, q = divmod(core, 4)
        out[b, :, q * HC:(q + 1) * HC, :] = results[core]["y"]
    return out


def kernel(input):
    from concourse.bass_utils import run_bass_kernel_spmd

    if "nc" not in _CACHED:
        _CACHED["nc"] = build_nc()
    in_maps = shard_inputs(input)
    res = run_bass_kernel_spmd(_CACHED["nc"], in_maps, list(range(8)))
    return assemble(res.results)


# revision 33
# speedup vs baseline: 1.3659x; 1.0191x over previous
"""Bilateral filter (7x7, dilation 1) Trainium2 Bass kernel, v3.

Problem: input [2, 18, 1024, 1024] f32.
  filterable f = input[:, :8]; params p = input[:, 8:]
  logw(tap) = -sum_c p_c^2 (fn_c - f_c)^2 - p8^2 dx^2 - p9^2 dy^2
  out_c = sum_taps exp(logw) fn_c / sum_taps exp(logw),  c < 3

Sharding: 8 cores x (one batch quarter of 256 rows + 3 halo rows), as in
the baseline.  Out-of-image taps are killed by a 1e18 sentinel pad (the
quadratic form then underflows exp to +0).

Per-core layout (bf16):  macro = 64 output rows x full width.
  tiles put (channel-pair c in {0,1}, row r in 0..63) on partitions and
  (segment g in 0..3, x) on the free axis, so channel ci = 2g + c.
  Row shifts -> 7 DMA'd tile variants F[oy]; column shifts -> free-axis
  offsets (DVE needs in0/in1 offsets with equal parity, so the center
  operand comes from F[3] (odd offset 3) for odd j and from a separate
  even-phase tile Fce (offset 2) for even j).

Per tap: d = Fsh - Fc (DVE), d2 = Square(d) (ACT), q = (p^2)*d2 (DVE or
GPSIMD, flat), then PE matmuls with a -1 row-selector lhsT contract the
channels of q into PSUM (2 taps per bank, M=64 at partitions 0/64), plus
one matmul with lhsT = -(dx^2, dy^2) selector against the (p8^2, p9^2)
tile for the spatial term.  w = Exp(PSUM) on ACT.  acc_c and wsum are
accumulated over all taps by PE matmuls with a +1 selector against
WF_c = w * G_c (gathered neighbor tiles) and w itself.
"""

import sys

if "/opt/trn_rl_repo" not in sys.path:
    sys.path.insert(0, "/opt/trn_rl_repo")

import numpy as np
import ml_dtypes

import concourse.bass as bass
import concourse.mybir as mybir
from concourse.bacc import Bacc
from concourse.tile import TileContext

FP32 = mybir.dt.float32
BF16 = mybir.dt.bfloat16
FP8 = mybir.dt.float8e4
ALU = mybir.AluOpType
ACTF = mybir.ActivationFunctionType

B, C_ALL, H, W = 2, 18, 1024, 1024
CF, CO = 8, 3
KS, RAD = 7, 3
HC = 256                      # output rows per core
HIN = HC + 2 * RAD            # slab rows (262)
WP = W + 2 * RAD              # padded slab width (1030)
MR = 64                       # macro rows
NM = HC // MR                 # 4 macros
SEG = 4                       # channel-pair segments per tile
SW = W + 8                    # segment width in F tiles (1032, even)
SENT = 9.0
BF = ml_dtypes.bfloat16

_CACHED = {}

# tap pairing: 48 non-center taps in 24 pairs + (center, dead)
_TAPS = [(i, j) for i in range(KS) for j in range(KS) if not (i == RAD and j == RAD)]
PAIRS = [(_TAPS[2 * k], _TAPS[2 * k + 1]) for k in range(24)]
CENTER_PAIR_IDX = 24          # slot0 = center tap (w=1), slot1 = dead (w=0)

N_CONST = 19                  # SELN, SELP, 16 spatial matrices, SHIFTC


def _seg(ap, width=SW):
    return ap.rearrange("p (s x) -> p s x", s=SEG)


def build_consts8():
    c = np.zeros((1, 128, 64), dtype=np.float32)
    r = np.arange(64)
    for g in range(2):
        c[0, g * 64 + r, r] = -64.0
    return c.astype(ml_dtypes.float8_e4m3)


def build_consts():
    """[18, 128, 64] bf16: row-selector and spatial lhsT matrices."""
    c = np.zeros((N_CONST, 128, 64), dtype=np.float32)
    r = np.arange(64)
    for g in range(2):
        c[0, g * 64 + r, r] = -1.0          # SELN
        c[1, g * 64 + r, r] = 1.0           # SELP
    for a in range(4):                       # dx2 index
        for b in range(4):                   # dy2 index
            m = 2 + a * 4 + b
            c[m, 0 * 64 + r, r] = -float([0, 1, 4, 9][a])
            c[m, 1 * 64 + r, r] = -float([0, 1, 4, 9][b])
    c[18, 64 + r, r] = 1.0                   # SHIFTC: out[m] = in[m + 64]
    return c.astype(BF)


D2IDX = [3, 2, 1, 0, 1, 2, 3]


def build_nc():
    nc = Bacc()
    fil = nc.dram_tensor("fil", [CF, HIN, WP], BF16, kind="ExternalInput")
    prm = nc.dram_tensor("prm", [10, HC, W], BF16, kind="ExternalInput")
    cst = nc.dram_tensor("cst", [N_CONST, 128, 64], BF16, kind="ExternalInput")
    cst8 = nc.dram_tensor("cst8", [1, 128, 64], FP8, kind="ExternalInput")
    y = nc.dram_tensor("y", [CO, HC, W], FP32, kind="ExternalOutput")

    with TileContext(nc) as tc:
        with (
            tc.tile_pool(name="const", bufs=1) as kpool,
            tc.tile_pool(name="fpool", bufs=1) as fpool,
            tc.tile_pool(name="dpool", bufs=3) as dpool,
            tc.tile_pool(name="wpool", bufs=2) as wpool,
            tc.tile_pool(name="opool", bufs=2) as opool,
            tc.tile_pool(name="ppool", bufs=4, space="PSUM") as ppool,
        ):
            CST = kpool.tile([128, N_CONST * 64], BF16, name="CST")
            for m in range(N_CONST):
                nc.sync.dma_start(out=CST[:, m * 64:(m + 1) * 64], in_=cst[m])
            CST8 = kpool.tile([128, 64], FP8, name="CST8")
            nc.sync.dma_start(out=CST8[:], in_=cst8[0])
            SELN = CST8[:]
            SELP = CST[:, 64:128]
            SHIFTC = CST[:, 18 * 64:19 * 64]

            def SPAT(i, j):
                m = 2 + D2IDX[j] * 4 + D2IDX[i]
                return CST[:, m * 64:(m + 1) * 64]

            for mi in range(NM):
                _macro(nc, tc, fil, prm, y, kpool, fpool, dpool, wpool, opool,
                       ppool, mi, SELN, SELP, SPAT, SHIFTC)
    nc.compile()
    return nc


def _macro(nc, tc, fil, prm, y, kpool, fpool, dpool, wpool, opool, ppool, mi,
           SELN, SELP, SPAT, SHIFTC):
    r0 = mi * MR

    # ---- F tiles: 7 row-shift variants + even-phase center ----
    F = []
    for oy in range(KS):
        Ft = fpool.tile([128, SEG * SW], BF16, tag=f"F{oy}", name=f"F{oy}_{mi}")
        for g in range(SEG):
            for c in range(2):
                nc.sync.dma_start(
                    out=_seg(Ft[:])[c * 64:(c + 1) * 64, g, 0:WP],
                    in_=fil[2 * g + c, r0 + oy:r0 + oy + MR, 0:WP])
        F.append(Ft)
    Fce = fpool.tile([128, SEG * SW], BF16, tag="Fce", name=f"Fce_{mi}")
    for g in range(SEG):
        for c in range(2):
            nc.sync.dma_start(
                out=_seg(Fce[:])[c * 64:(c + 1) * 64, g, 0:WP - 1],
                in_=fil[2 * g + c, r0 + RAD:r0 + RAD + MR, 1:WP])
    # center views at offset 3 (odd phase, in F[3]) and offset 2 (even, Fce)
    Fc_o = _seg(F[RAD][:])[:, :, 3:3 + W]
    Fc_e = _seg(Fce[:])[:, :, 2:2 + W]

    # ---- R = p_c^2 (bf16), SXY = (p8^2, p9^2) ----
    Rw = fpool.tile([128, SEG * W], BF16, tag="R", name=f"R_{mi}")
    for g in range(SEG):
        for c in range(2):
            nc.sync.dma_start(
                out=Rw[c * 64:(c + 1) * 64, g * W:(g + 1) * W],
                in_=prm[2 * g + c, r0:r0 + MR, :])
    SXY = fpool.tile([128, W], BF16, tag="SXY", name=f"SXY_{mi}")
    for k in range(2):
        nc.sync.dma_start(out=SXY[k * 64:(k + 1) * 64, :],
                          in_=prm[8 + k, r0:r0 + MR, :])

    # ---- PSUM accumulators: acc0,acc1 | acc2,wsum  (x2 N-halves) ----
    ACC = [ppool.tile([128, 512], FP32, tag="ACC", name=f"ACC{k}_{mi}")
           for k in range(4)]   # [A_h0, A_h1, B_h0, B_h1]

    nq = 0
    for pi, pair in enumerate(PAIRS + [((RAD, RAD), None)]):
        w_t = wpool.tile([128, W], BF16, tag="w", name=f"w_{mi}_{pi}", bufs=3)
        if pi == CENTER_PAIR_IDX:
            nc.gpsimd.memset(w_t[0:64, :], 1.0)
            nc.gpsimd.memset(w_t[64:128, :], 0.0)
            g_taps = (pair[0], pair[0])
        else:
            SP = [ppool.tile([128, 512], FP32, tag="SP", name=f"SP{h}_{mi}_{pi}",
                             bufs=4) for h in range(2)]
            d2s = []
            for sl, (i, j) in enumerate(pair):
                Fc = Fc_o if (j % 2 == 1) else Fc_e
                d = dpool.tile([128, SEG * W], BF16, tag="d",
                               name=f"d_{mi}_{pi}_{sl}", bufs=2)
                dv = d[:].rearrange("p (s x) -> p s x", s=SEG)
                nc.vector.tensor_sub(dv, _seg(F[i][:])[:, :, j:j + W], Fc)
                t8 = dpool.tile([128, SEG * W], BF16, tag="t8",
                                name=f"t8_{mi}_{pi}_{sl}", bufs=2)
                nc.vector.tensor_mul(t8[:], Rw[:], d[:])
                d2 = dpool.tile([128, SEG * W], FP8, tag="d2",
                                name=f"d2_{mi}_{pi}_{sl}", bufs=2)
                nc.scalar.activation(d2[:], t8[:], ACTF.Square)
                d2s.append(d2)
            for sl, (i, j) in enumerate(pair):
                d2 = d2s[sl]
                nq += 1
                pos = slice(sl * 64, sl * 64 + 64)
                for h in range(2):
                    for g in range(SEG):
                        nc.tensor.matmul(
                            SP[h][pos, :], SELN,
                            d2[:, g * W + h * 512:g * W + h * 512 + 512],
                            start=(g == 0), stop=False, skip_group_check=True)
                    nc.tensor.matmul(
                        SP[h][pos, :], SPAT(i, j),
                        SXY[:, h * 512:h * 512 + 512],
                        start=False, stop=True, skip_group_check=True)
            for h in range(2):
                nc.scalar.activation(w_t[:, h * 512:h * 512 + 512], SP[h][:],
                                     ACTF.Exp)
            g_taps = pair

        # ---- gather tiles + weighted accumulation ----
        first = pi == 0
        last = pi == CENTER_PAIR_IDX
        G3 = wpool.tile([128, CO * W], BF16, tag="G3", name=f"G3_{mi}_{pi}",
                        bufs=3)
        for sl in range(2):
            i, j = g_taps[sl]
            for c in range(CO):
                nc.sync.dma_start(
                    out=G3[sl * 64:sl * 64 + 64, c * W:(c + 1) * W],
                    in_=fil[c, r0 + i:r0 + i + MR, j:j + W])
        WFs = []
        for c in range(CO):
            WF = wpool.tile([128, W], BF16, tag=f"WF{c}",
                            name=f"WF{c}_{mi}_{pi}", bufs=3)
            nc.vector.tensor_mul(WF[:], w_t[:], G3[:, c * W:(c + 1) * W])
            WFs.append(WF)
        for h in range(2):
            hs = slice(h * 512, h * 512 + 512)
            nc.tensor.matmul(ACC[h][0:64, :], SELP, WFs[0][:, hs],
                             start=first, stop=last, skip_group_check=True)
            nc.tensor.matmul(ACC[h][64:128, :], SELP, WFs[1][:, hs],
                             start=first, stop=last, skip_group_check=True)
            nc.tensor.matmul(ACC[2 + h][0:64, :], SELP, WFs[2][:, hs],
                             start=first, stop=last, skip_group_check=True)
            nc.tensor.matmul(ACC[2 + h][64:128, :], SELP, w_t[:, hs],
                             start=first, stop=last, skip_group_check=True)

    # ---- out = acc / wsum ----
    # wsum lives at partitions 64:128 of ACC[2+h]; DMA-replicate it to both
    # partition halves so every elementwise op sees matching base partitions.
    for h in range(2):
        hs = slice(h * 512, h * 512 + 512)
        AS = opool.tile([128, 512], BF16, tag="as", name=f"as_{mi}_{h}")
        nc.scalar.copy(AS[:], ACC[2 + h][:])
        TMP = ppool.tile([128, 512], FP32, tag="SP", name=f"wtmp_{mi}_{h}",
                         bufs=4)
        nc.tensor.matmul(TMP[0:64, :], SHIFTC, AS[:], start=True, stop=True,
                         skip_group_check=True)
        nc.tensor.matmul(TMP[64:128, :], SHIFTC, AS[:], start=True, stop=True,
                         skip_group_check=True)
        RW = opool.tile([128, 512], FP32, tag="rw", name=f"rw_{mi}_{h}")
        nc.vector.reciprocal(RW[:], TMP[:])
        o = opool.tile([128, 512], FP32, tag="o", name=f"o_{mi}_{h}")
        o2 = opool.tile([128, 512], FP32, tag="o2", name=f"o2_{mi}_{h}")
        nc.vector.tensor_mul(o[0:64, :], ACC[h][0:64, :], RW[0:64, :])
        nc.vector.tensor_mul(o[64:128, :], ACC[h][64:128, :], RW[64:128, :])
        nc.vector.tensor_mul(o2[0:64, :], ACC[2 + h][0:64, :], RW[0:64, :])
        nc.sync.dma_start(out=y[0, r0:r0 + MR, hs], in_=o[0:64, :])
        nc.sync.dma_start(out=y[1, r0:r0 + MR, hs], in_=o[64:128, :])
        nc.sync.dma_start(out=y[2, r0:r0 + MR, hs], in_=o2[0:64, :])


def shard_inputs(input):
    """input [2,18,1024,1024] f32 -> per-core {fil, prm, cst} bf16."""
    input = np.asarray(input, dtype=np.float32)
    cst = build_consts()
    cst8v = build_consts8()
    fil_all = input[:, :CF]
    prm_all = input[:, CF:]
    in_maps = []
    for core in range(8):
        b, q = divmod(core, 4)
        r0 = q * HC
        slab = np.full((CF, HIN, WP), SENT, dtype=np.float32)
        s_lo = max(r0 - RAD, 0)
        s_hi = min(r0 + HC + RAD, H)
        slab[:, s_lo - (r0 - RAD):s_hi - (r0 - RAD), RAD:RAD + W] = \
            fil_all[b, :, s_lo:s_hi, :]
        prm = prm_all[b, :, r0:r0 + HC, :].astype(np.float32).copy()
        prm[0:8] = np.abs(prm[0:8]) * 0.125
        prm[8:10] = prm[8:10] * prm[8:10]
        in_maps.append({
            "fil": np.ascontiguousarray(slab.astype(BF)),
            "prm": np.ascontiguousarray(prm.astype(BF)),
            "cst": cst,
            "cst8": cst8v,
        })
    return in_maps


def assemble(results):
    out = np.empty((B, CO, H, W), dtype=np.float32)
    for core in range(8):
        b, q = divmod(core, 4)
        out[b, :, q * HC:(q + 1) * HC, :] = results[core]["y"]
    return out


def kernel(input):
    from concourse.bass_utils import run_bass_kernel_spmd

    if "nc" not in _CACHED:
        _CACHED["nc"] = build_nc()
    in_maps = shard_inputs(input)
    res = run_bass_kernel_spmd(_CACHED["nc"], in_maps, list(range(8)))
    return assemble(res.results)
